# revision 1
# baseline (speedup 1.0000x reference)
"""Trainium2 Bass kernel for nn_DepthEstimationNet (vq_codebook).

reference:  d = x.reshape(B, S);  ratio[b,i,j] = d[b,i] * (1/d[b,j])
            out[b,i,j] = inv[searchsorted(q, ratio, side='right')]
shapes:     x [8,1,48,48] -> out [8, 2304, 2304] fp32 (~170 MB)

Strategy (data-parallel over batch, one batch per NeuronCore):
  - host computes recip = fl32(1/d) per batch (bit-identical to the
    reference's fp32 divide) and replicates it across 128 SBUF partitions.
  - per 128-row tile: v = d_col * recip (same fp32 rounding as the
    reference ratio), then a 40-step select-chain
        s = select(v >= q_k, inv[k+1], s)
    via a custom DVE op. Exact: compares are exact, values are copied.
  - row tiles are processed in groups of 3-4 with one wide DVE op per
    chain step ([128, W*2304]) to amortize per-instruction overhead.
  - q/inv are instruction immediates (same for all cores -> SPMD NEFF).
"""
import numpy as np

S = 2304          # 48*48
P = 128           # partitions
NT = S // P       # 18 row tiles per batch
NB = 40           # thresholds
B = 8             # batch == cores
GROUPS = (4, 4, 4, 3, 3)   # tile-group widths, sum = NT
WMAX = max(GROUPS)

_CACHE = {}


def _register_ops():
    import dataclasses
    import concourse.dve_ops as dve_ops_mod
    from concourse.dve_spec import Spec, Src0, Src1, C0, C1, C2, select
    from concourse.dve_ops import DveOp, OPS
    from concourse.dve_table_gen import dve_ver_for

    def reg(name, spec):
        for op in OPS:
            if op.name == name:
                return op
        op = DveOp(name, spec, subdim=False, uops_sha={})
        OPS.append(op)
        dve_ops_mod._SUB_OPCODE_FOR_NAME[name] = (
            dve_ops_mod._CUSTOM_DVE_ROW_BASE + len(OPS) - 1
        )
        assert dve_ops_mod._SUB_OPCODE_FOR_NAME[name] < 0x20
        dve_ops_mod.CUSTOM_DVE_SPECS[name] = spec
        ver = dve_ver_for("TRN2")
        try:
            op.compile(ver)
            return op
        except ValueError as e:
            import re
            m = re.search(r'uops_sha\["' + ver + r'"\]="([0-9a-f]+)"', str(e))
            assert m, f"no sha in: {e}"
            op2 = dataclasses.replace(op, uops_sha={ver: m.group(1)})
            OPS[OPS.index(op)] = op2
            return op2

    selchain = reg("ANT_SELCHAIN", Spec(body=select(Src0 >= C0, C1, Src1)))
    selinit = reg("ANT_SELINIT", Spec(body=select(Src0 >= C0, C1, C2)))
    return selchain, selinit


def _build_nc(q, inv, repeat=1, tiny_out=False):
    import concourse.bass as bass
    import concourse.mybir as mybir

    SELCHAIN, SELINIT = _register_ops()
    f32 = mybir.dt.float32

    nc = bass.Bass()
    r_in = nc.declare_dram_parameter("recipb", [P, S], f32, isOutput=False)
    d_in = nc.declare_dram_parameter("dcol", [P, NT], f32, isOutput=False)
    out_shape = [P, 8] if tiny_out else [S, S]
    y_out = nc.declare_dram_parameter("out", out_shape, f32, isOutput=True)

    NG = len(GROUPS)
    with (
        nc.sbuf_tensor("rb", [P, S], f32) as rb,
        nc.sbuf_tensor("dc", [P, NT], f32) as dc,
        nc.sbuf_tensor("v", [P, WMAX * S], f32) as v,
        nc.sbuf_tensor("x", [P, WMAX * S], f32) as x,
        nc.sbuf_tensor("y0", [P, WMAX * S], f32) as y0,
        nc.sbuf_tensor("y1", [P, WMAX * S], f32) as y1,
        nc.Block() as block,
        nc.semaphore("in_sem") as in_sem,
        nc.semaphore("grp_done") as grp_done,
        nc.semaphore("out_sem") as out_sem,
    ):
        ys = (y0, y1)

        @block.sync
        def _(sync):
            sync.dma_start(out=rb[:], in_=r_in[:]).then_inc(in_sem, 16)
            sync.dma_start(out=dc[:], in_=d_in[:]).then_inc(in_sem, 16)
            if tiny_out:
                sync.wait_ge(grp_done, NG * repeat)
                sync.dma_start(out=y_out[:], in_=y0[:, 0:8]).then_inc(out_sem, 16)
                sync.wait_ge(out_sem, 16)
            else:
                row0 = 0
                for g, W in enumerate(GROUPS):
                    sync.wait_ge(grp_done, g + 1)
                    dst = y_out[row0:row0 + W * P, :].rearrange(
                        "(w p) s -> p w s", p=P
                    )
                    src = ys[g % 2][:, 0:W * S].rearrange(
                        "p (w s) -> p w s", s=S
                    )
                    sync.dma_start(out=dst, in_=src).then_inc(out_sem, 16)
                    row0 += W * P
                sync.wait_ge(out_sem, 16 * NG)

        @block.vector
        def _(vector):
            vector.wait_ge(in_sem, 32)
            import contextlib
            rep_ctx = (
                vector.Fori(0, repeat) if repeat > 1 else contextlib.nullcontext()
            )
            with rep_ctx:
                t0 = 0
                for g, W in enumerate(GROUPS):
                    M = W * S
                    yv = ys[g % 2]
                    if not tiny_out and g >= 2:
                        vector.wait_ge(out_sem, 16 * (g - 1))
                    for w in range(W):
                        vector.tensor_scalar_mul(
                            v[:, w * S:(w + 1) * S], rb[:], dc[:, t0 + w:t0 + w + 1]
                        )
                    vector._custom_dve(
                        SELINIT, out=x[:, 0:M], in0=v[:, 0:M],
                        s0=float(q[0]), s1=float(inv[1]), imm2=float(inv[0]),
                    )
                    cur = x[:, 0:M]
                    for k in range(1, NB):
                        dst = yv[:, 0:M] if k % 2 == 1 else x[:, 0:M]
                        vector._custom_dve(
                            SELCHAIN, out=dst, in0=v[:, 0:M], in1=cur,
                            s0=float(q[k]), s1=float(inv[k + 1]),
                        )
                        cur = dst
                    assert (NB - 1) % 2 == 1  # final landed in yv
                    vector.engine_nop().then_inc(grp_done, 1)
                    t0 += W

    from concourse.library_overlay import lower_extended_insts
    lower_extended_insts(nc)
    return nc


def _in_maps(x, q, inv):
    d = x.reshape(B, S).astype(np.float32)
    recip = (np.float32(1.0) / d).astype(np.float32)
    maps = []
    for b in range(B):
        maps.append({
            "recipb": np.ascontiguousarray(np.broadcast_to(recip[b], (P, S))),
            "dcol": np.ascontiguousarray(d[b].reshape(NT, P).T),
        })
    return maps


def kernel(x, q, inv):
    x = np.asarray(x, dtype=np.float32)
    q = np.asarray(q, dtype=np.float32)
    inv = np.asarray(inv, dtype=np.float32)
    assert x.shape == (B, 1, 48, 48)

    key = (q.tobytes(), inv.tobytes())
    if key not in _CACHE:
        _CACHE[key] = _build_nc(q, inv)
    nc = _CACHE[key]

    from concourse.bass_utils import run_bass_kernel_spmd
    res = run_bass_kernel_spmd(nc, _in_maps(x, q, inv), list(range(B)))
    out = np.stack([res.results[b]["out"] for b in range(B)], axis=0)
    return out

